# revision 1
# baseline (speedup 1.0000x reference)
"""Chamfer distance kernel for Trainium2 — v3 (bf16-domain min + host repair).

v2 hit the DVE 1x floor: an f32 min-scan from PSUM (2.26us/2048) plus a
bf16 position-scan (1x) per block. v3 moves the min-scan into the bf16
domain where tensor_scalar runs at 2x:

- ACT copies each PSUM group to a bf16 strip (same cost as v2's Exp).
- bf16 rounding is monotone, so min(bf16(d)) == bf16(min(d)) EXACTLY; the
  chained DVE tensor_scalar(min, accum) now reads bf16 SBUF at 2x.
- max_index queries the bf16 block-min over the raw bf16 strips: the first
  position whose bf16 value equals the bf16 min. If that bucket holds only
  ONE element, this is exactly np.argmin of the f32 matrix.
- Repeated query slots return successive occurrences (slot1 = second match,
  0xFFFFFFFF if none), so multi-match rows (~1% - where bf16 buckets
  collide at the min) are detected for FREE; the host recomputes those few
  rows exactly in f32 numpy (exact argmin AND exact dist).
- dist for clean rows = bf16(min d), rel err ~1e-3 << the 2e-2 gate.
"""

import numpy as np
import ml_dtypes

import concourse.bacc as bacc
import concourse.mybir as mybir
from concourse import tile
from concourse.bass_utils import run_bass_kernel_spmd

F32 = mybir.dt.float32
BF16 = mybir.dt.bfloat16
U32 = mybir.dt.uint32
AF = mybir.ActivationFunctionType
ALU = mybir.AluOpType

BF = ml_dtypes.bfloat16
MISS = np.uint32(0xFFFFFFFF)

_PROGRAM_CACHE = {}


def _build_program(n_pts=8192, n_cores=8, repeat=1):
    key = (n_pts, n_cores, repeat)
    if key in _PROGRAM_CACHE:
        return _PROGRAM_CACHE[key]

    NB = n_pts // 128
    NG = 4
    GW = n_pts // NG

    nc = bacc.Bacc("TRN2", target_bir_lowering=False, debug=False,
                   num_devices=n_cores)
    uu = nc.dram_tensor("uu", [96, n_pts], BF16, kind="ExternalInput")
    s1o = nc.dram_tensor("s1", [128, NB * NG], F32, kind="ExternalOutput")
    x1o = nc.dram_tensor("x1", [128, NB * 16], U32, kind="ExternalOutput")
    s2o = nc.dram_tensor("s2", [128, NB * NG], F32, kind="ExternalOutput")
    x2o = nc.dram_tensor("x2", [128, NB * 16], U32, kind="ExternalOutput")

    with tile.TileContext(nc) as tc:
        with tc.tile_pool(name="persist", bufs=1) as persist:
            U1 = persist.tile([56, n_pts], BF16, tag="U1")
            U2 = persist.tile([56, n_pts], BF16, tag="U2")
            big = persist.tile([128, 1], F32, tag="big")
            sacc1 = persist.tile([128, NB * NG], F32, tag="sacc1")
            sacc2 = persist.tile([128, NB * NG], F32, tag="sacc2")
            xacc1 = persist.tile([128, NB * 16], U32, tag="xacc1")
            xacc2 = persist.tile([128, NB * 16], U32, tag="xacc2")

            nc.vector.memset(big[:], 3.4e38)
            nc.sync.dma_start(U1[0:24, :], uu.ap()[0:24, :])
            nc.sync.dma_start(U1[32:56, :], uu.ap()[24:48, :])
            nc.sync.dma_start(U2[0:24, :], uu.ap()[72:96, :])
            nc.sync.dma_start(U2[32:56, :], uu.ap()[48:72, :])

            dirs = ((U1, U2, 0, sacc1, xacc1),
                    (U2, U1, 32, sacc2, xacc2))

            with tc.tile_pool(name="psum", bufs=2, space="PSUM") as pspool, \
                 tc.tile_pool(name="tb", bufs=4) as tbpool, \
                 tc.tile_pool(name="jk", bufs=2) as jkpool, \
                 tc.tile_pool(name="q8", bufs=4) as q8pool:
                for _ in range(repeat):
                    pend = [None, None]

                    def emit_block(di, nb):
                        lhsU, rhsU, base, sacc, xacc = dirs[di]
                        tb = tbpool.tile([128, n_pts], BF16, tag="tb")
                        lhs = lhsU[base:base + 24, nb * 128:(nb + 1) * 128]
                        for g in range(NG - 1, -1, -1):
                            ps = pspool.tile([128, GW], F32, tag="ps")
                            for q in range(GW // 512):
                                c0 = g * GW + q * 512
                                nc.tensor.matmul(
                                    ps[:, q * 512:(q + 1) * 512],
                                    lhs,
                                    rhsU[base:base + 24, c0:c0 + 512],
                                    start=True, stop=True)
                            c = nb * NG + g
                            # bf16 value strip (monotone cast) frees PSUM
                            nc.scalar.activation(
                                tb[:, g * GW:(g + 1) * GW], ps[:], AF.Copy)
                            sprev = (big[:, 0:1] if g == NG - 1
                                     else sacc[:, c + 1:c + 2])
                            # chained block-min on bf16 SBUF (fast mode);
                            # elementwise out goes to a junk tile so the
                            # value strip stays intact for max_index
                            jk = jkpool.tile([128, GW], BF16, tag="jk")
                            nc.vector.tensor_scalar(
                                out=jk[:],
                                in0=tb[:, g * GW:(g + 1) * GW],
                                scalar1=sprev, scalar2=None,
                                op0=ALU.min, op1=ALU.min,
                                accum_out=sacc[:, c:c + 1])
                        # bf16 query = block min (exactly representable)
                        q8t = q8pool.tile([128, 8], BF16, tag="q8t")
                        nc.scalar.activation(
                            q8t[:],
                            sacc[:, nb * NG:nb * NG + 1].broadcast_to((128, 8)),
                            AF.Copy)
                        pend[di] = (tb, q8t, nb)

                    def emit_maxidx(di):
                        if pend[di] is None:
                            return
                        xacc = dirs[di][4]
                        ptb, pq, pnb = pend[di]
                        nc.vector.max_index(
                            xacc[:, pnb * 16:pnb * 16 + 8], pq[:],
                            ptb[:, 0:2 * GW])
                        nc.vector.max_index(
                            xacc[:, pnb * 16 + 8:pnb * 16 + 16], pq[:],
                            ptb[:, 2 * GW:4 * GW])
                        pend[di] = None

                    for nb in range(NB):
                        emit_block(0, nb)
                        emit_maxidx(1)
                        emit_block(1, nb)
                        emit_maxidx(0)
                    emit_maxidx(1)

            nc.sync.dma_start(s1o.ap(), sacc1[:])
            nc.sync.dma_start(x1o.ap(), xacc1[:])
            nc.sync.dma_start(s2o.ap(), sacc2[:])
            nc.sync.dma_start(x2o.ap(), xacc2[:])

    nc.compile()
    _PROGRAM_CACHE[key] = nc
    return nc


def _split3(v):
    h = v.astype(BF).astype(np.float32)
    r = (v - h).astype(np.float32)
    m = r.astype(BF).astype(np.float32)
    l = (r - m).astype(BF).astype(np.float32)
    return h, m, l


def _forms(xyz):
    """[N,3] f32 -> (A, B) [24, N] bf16 triple-split homogeneous forms."""
    x = np.ascontiguousarray(xyz.T).astype(np.float32)
    n = (x * x).sum(0, dtype=np.float32)[None, :]
    s = (-2.0 * x).astype(np.float32)
    sh, sm, sl = _split3(s)
    xh, xm, xl = _split3(x)
    nh, nm, nl = _split3(n)
    ones = np.ones_like(n)
    A = np.concatenate([sh, sh, sm, sh, sl, sm, ones, ones, ones,
                        nh, nm, nl]).astype(BF)
    Bf = np.concatenate([xh, xm, xh, xl, xh, xm, nh, nm, nl,
                         ones, ones, ones]).astype(BF)
    return A, Bf


def kernel(xyz1: np.ndarray, xyz2: np.ndarray, repeat: int = 1):
    xyz1 = np.asarray(xyz1, dtype=np.float32)
    xyz2 = np.asarray(xyz2, dtype=np.float32)
    B, N, _ = xyz1.shape
    M = xyz2.shape[1]
    assert B == 8 and N == 8192 and M == 8192, (B, N, M)

    nc = _build_program(N, B, repeat)

    in_maps = []
    for b in range(B):
        A1, B1 = _forms(xyz1[b])
        A2, B2 = _forms(xyz2[b])
        in_maps.append({"uu": np.concatenate([A1, B1, A2, B2])})
    res = run_bass_kernel_spmd(nc, in_maps, list(range(B)))

    NB = N // 128
    dist1 = np.empty((B, N), np.float32)
    dist2 = np.empty((B, M), np.float32)
    idx1 = np.empty((B, N), np.int32)
    idx2 = np.empty((B, M), np.int32)
    for b in range(B):
        r = res.results[b]
        for dirn, (s_name, x_name, dist, idx, Xq, Xc) in enumerate((
                ("s1", "x1", dist1, idx1, xyz1[b], xyz2[b]),
                ("s2", "x2", dist2, idx2, xyz2[b], xyz1[b]))):
            s = np.asarray(r[s_name])
            xi = np.asarray(r[x_name])
            vmin = s[:, 0::4]                   # [128, NB] block min (bf16)
            lo0, lo1 = xi[:, 0::16], xi[:, 1::16]
            hi0, hi1 = xi[:, 8::16], xi[:, 9::16]
            lo_hit = lo0 != MISS
            ix = np.where(lo_hit, lo0, hi0 + 4096).astype(np.int64)
            # multi-match rows: second occurrence anywhere
            multi = np.where(lo_hit, (lo1 != MISS) | (hi0 != MISS),
                             hi1 != MISS)
            d_full = np.maximum(vmin, 0.0).T.reshape(-1)
            i_full = ix.T.reshape(-1)
            flag = multi.T.reshape(-1)
            rows = np.nonzero(flag)[0]
            if rows.size:
                # exact f32 repair for bf16-collision rows
                q = Xq[rows]                                    # [R, 3]
                d = (q * q).sum(1)[:, None] + (Xc * Xc).sum(1)[None, :] \
                    - 2.0 * (q @ Xc.T)
                d = np.maximum(d.astype(np.float32), 0.0)
                i_full[rows] = d.argmin(1)
                d_full[rows] = d.min(1)
            dist[b] = d_full
            idx[b] = i_full.astype(np.int32)
    return dist1, dist2, idx1, idx2



# revision 6
# speedup vs baseline: 1.3629x; 1.3629x over previous
"""Chamfer distance kernel for Trainium2 — v4 (fold-tree + host candidate refine).

v3 was DVE-bound: per 128-row block, two max_index scans over the full
8192-wide bf16 strip (1x mode) cost ~8.7us of the ~11.2us DVE budget.

v4 removes max_index entirely:
- The 8192 PSUM f32 columns per (block, direction) are evacuated by a
  split: 1 DVE tensor_tensor min (evacuates 2048 cols as a 1024-wide
  fold-2 strip) + 3 ACT copies (bf16 strips), balancing the two engines.
- A DVE bf16 fold tree (tensor_tensor at 2x, tensor_reduce tail) folds
  everything to 64 positions per (block, direction); each final position
  covers a fixed, host-known set of 128 columns.
- Host: per row, argmin over the 64 bf16 position-minima (+ bf16 ties),
  then EXACT f32 recompute of the <=128 candidate columns -> exact idx
  and exact dist for every row. bf16 rounding is monotone, so the column
  achieving the true f32 row-min always lands in a tied-min position.
"""

import numpy as np
import ml_dtypes

import concourse.bacc as bacc
import concourse.mybir as mybir
from concourse import tile
from concourse.bass_utils import run_bass_kernel_spmd

F32 = mybir.dt.float32
BF16 = mybir.dt.bfloat16
AF = mybir.ActivationFunctionType
ALU = mybir.AluOpType
AX = mybir.AxisListType

BF = ml_dtypes.bfloat16

_PROGRAM_CACHE = {}


def _build_program(n_pts=8192, n_cores=8, repeat=1):
    key = (n_pts, n_cores, repeat)
    if key in _PROGRAM_CACHE:
        return _PROGRAM_CACHE[key]

    NB = n_pts // 128          # 64 row blocks
    TW = 2048                  # psum tile width (4 banks)
    NT = n_pts // TW           # 4 psum tiles per (block, dir)
    NPOS = 64                  # final positions per (block, dir)

    nc = bacc.Bacc("TRN2", target_bir_lowering=False, debug=False,
                   num_devices=n_cores)
    uu = nc.dram_tensor("uu", [96, n_pts], BF16, kind="ExternalInput")
    r1o = nc.dram_tensor("r1", [128, NB * NPOS], BF16, kind="ExternalOutput")
    r2o = nc.dram_tensor("r2", [128, NB * NPOS], BF16, kind="ExternalOutput")

    with tile.TileContext(nc) as tc:
        with tc.tile_pool(name="persist", bufs=1) as persist:
            U1 = persist.tile([56, n_pts], BF16, tag="U1")
            U2 = persist.tile([56, n_pts], BF16, tag="U2")
            racc1 = persist.tile([128, NB * NPOS], BF16, tag="racc1")
            racc2 = persist.tile([128, NB * NPOS], BF16, tag="racc2")

            nc.sync.dma_start(U1[0:24, :], uu.ap()[0:24, :])
            nc.sync.dma_start(U1[32:56, :], uu.ap()[24:48, :])
            nc.sync.dma_start(U2[0:24, :], uu.ap()[72:96, :])
            nc.sync.dma_start(U2[32:56, :], uu.ap()[48:72, :])

            dirs = ((U1, U2, 0, racc1),
                    (U2, U1, 32, racc2))

            with tc.tile_pool(name="psum", bufs=2, space="PSUM") as pspool, \
                 tc.tile_pool(name="tt", bufs=2) as tpool, \
                 tc.tile_pool(name="zz", bufs=2) as zpool, \
                 tc.tile_pool(name="vv", bufs=2) as vpool, \
                 tc.tile_pool(name="ww", bufs=2) as wpool, \
                 tc.tile_pool(name="xx", bufs=2) as xpool, \
                 tc.tile_pool(name="yy", bufs=2) as ypool:
                for _ in range(repeat):
                    for nb in range(NB):
                        for di in (0, 1):
                            lhsU, rhsU, base, racc = dirs[di]
                            lhs = lhsU[base:base + 24,
                                       nb * 128:(nb + 1) * 128]
                            T = tpool.tile([128, 3 * TW], BF16, tag="T")
                            C = zpool.tile([128, TW], BF16, tag="C")
                            for t in range(NT):
                                ps = pspool.tile([128, TW], F32, tag="ps")
                                for q in range(TW // 512):
                                    c0 = t * TW + q * 512
                                    nc.tensor.matmul(
                                        ps[:, q * 512:(q + 1) * 512],
                                        lhs,
                                        rhsU[base:base + 24, c0:c0 + 512],
                                        start=True, stop=True)
                                if t < 3:
                                    nc.scalar.activation(
                                        T[:, t * TW:(t + 1) * TW],
                                        ps[:], AF.Copy)
                                else:
                                    # DVE evac: fold tile3 (PSUM) with
                                    # tile2's SBUF strip (one PSUM input)
                                    nc.vector.tensor_tensor(
                                        C[:], ps[:], T[:, 2 * TW:3 * TW],
                                        ALU.min)
                            # bf16 fold tree (DVE 2x) down to 64 positions
                            V = vpool.tile([128, TW], BF16, tag="V")
                            nc.vector.tensor_tensor(
                                V[:], T[:, 0:TW], T[:, TW:2 * TW], ALU.min)
                            W = wpool.tile([128, TW], BF16, tag="W")
                            nc.vector.tensor_tensor(
                                W[:], C[:], V[:], ALU.min)
                            X = xpool.tile([128, 1024], BF16, tag="X")
                            nc.vector.tensor_tensor(
                                X[:], W[:, 0:1024], W[:, 1024:2048], ALU.min)
                            Y = ypool.tile([128, 512], BF16, tag="Y")
                            nc.vector.tensor_tensor(
                                Y[:], X[:, 0:512], X[:, 512:1024], ALU.min)
                            nc.vector.tensor_reduce(
                                racc[:, nb * NPOS:(nb + 1) * NPOS],
                                Y[:].rearrange("p (s k) -> p s k", k=8),
                                AX.X, ALU.min)

            nc.sync.dma_start(r1o.ap(), racc1[:])
            nc.sync.dma_start(r2o.ap(), racc2[:])

    nc.compile()
    _PROGRAM_CACHE[key] = nc
    return nc


def _split3(v):
    h = v.astype(BF).astype(np.float32)
    r = (v - h).astype(np.float32)
    m = r.astype(BF).astype(np.float32)
    l = (r - m).astype(BF).astype(np.float32)
    return h, m, l


def _forms(xyz):
    """[N,3] f32 -> (A, B) [24, N] bf16 triple-split homogeneous forms."""
    x = np.ascontiguousarray(xyz.T).astype(np.float32)
    n = (x * x).sum(0, dtype=np.float32)[None, :]
    s = (-2.0 * x).astype(np.float32)
    sh, sm, sl = _split3(s)
    xh, xm, xl = _split3(x)
    nh, nm, nl = _split3(n)
    ones = np.ones_like(n)
    A = np.concatenate([sh, sh, sm, sh, sl, sm, ones, ones, ones,
                        nh, nm, nl]).astype(BF)
    Bf = np.concatenate([xh, xm, xh, xl, xh, xm, nh, nm, nl,
                         ones, ones, ones]).astype(BF)
    return A, Bf


def _fold_maps(n_pts=8192):
    """colmap[col] = final position (0..63) within a (block, dir);
    cands[s] = ascending array of the 128 columns folded into position s.

    Fold structure: W[m] = min over cols congruent to m (mod 2048), then
    halvings to 512 and a reduce over 8 -> position s = (col % 512) // 8."""
    cols = np.arange(n_pts)
    s = (cols % 512) // 8
    cands = np.empty((64, 128), np.int64)
    for p in range(64):
        cc = np.nonzero(s == p)[0]
        assert cc.size == 128
        cands[p] = cc
    return s, cands


_COLMAP, _CANDS = _fold_maps()


def _refine(R, Xq, Xc):
    """R: [128, NB*64] bf16 position minima for one (batch, dir).
    Xq: query points [N, 3], Xc: candidate cloud [M, 3].
    Returns exact (dist [N], idx [N]) via f32 recompute of candidates."""
    NB = R.shape[1] // 64
    N = NB * 128
    # row n = b*128 + p  ->  vals[n] = R[p, b*64:(b+1)*64]
    vals = np.asarray(R).reshape(128, NB, 64).transpose(1, 0, 2).reshape(N, 64)
    vf = vals.astype(np.float32)
    m = vf.min(1)
    smin = vf.argmin(1)
    nties = (vf == m[:, None]).sum(1)
    cands = _CANDS[smin]                                   # [N, 128]
    nc2 = (Xc * Xc).sum(1)                                 # [M]
    nq = (Xq * Xq).sum(1)                                  # [N]
    cpts = Xc[cands]                                       # [N, 128, 3]
    d = nq[:, None] + nc2[cands] \
        - 2.0 * np.einsum('nd,nkd->nk', Xq, cpts)
    d = np.maximum(d.astype(np.float32), 0.0)
    loc = d.argmin(1)
    idx = cands[np.arange(N), loc]
    dist = d[np.arange(N), loc]
    # rows where several positions tie at the bf16 min: search their union
    rows = np.nonzero(nties > 1)[0]
    for r in rows:
        ss = np.nonzero(vf[r] == m[r])[0]
        cc = np.sort(np.concatenate([_CANDS[s] for s in ss]))
        dd = nq[r] + nc2[cc] - 2.0 * (Xc[cc] @ Xq[r])
        dd = np.maximum(dd.astype(np.float32), 0.0)
        l = dd.argmin()
        idx[r] = cc[l]
        dist[r] = dd[l]
    return dist, idx.astype(np.int32)


def kernel(xyz1: np.ndarray, xyz2: np.ndarray, repeat: int = 1):
    xyz1 = np.asarray(xyz1, dtype=np.float32)
    xyz2 = np.asarray(xyz2, dtype=np.float32)
    B, N, _ = xyz1.shape
    M = xyz2.shape[1]
    assert B == 8 and N == 8192 and M == 8192, (B, N, M)

    nc = _build_program(N, B, repeat)

    in_maps = []
    for b in range(B):
        A1, B1 = _forms(xyz1[b])
        A2, B2 = _forms(xyz2[b])
        in_maps.append({"uu": np.concatenate([A1, B1, A2, B2])})
    res = run_bass_kernel_spmd(nc, in_maps, list(range(B)))

    dist1 = np.empty((B, N), np.float32)
    dist2 = np.empty((B, M), np.float32)
    idx1 = np.empty((B, N), np.int32)
    idx2 = np.empty((B, M), np.int32)
    for b in range(B):
        r = res.results[b]
        dist1[b], idx1[b] = _refine(np.asarray(r["r1"]), xyz1[b], xyz2[b])
        dist2[b], idx2[b] = _refine(np.asarray(r["r2"]), xyz2[b], xyz1[b])
    return dist1, dist2, idx1, idx2
